# revision 32
# baseline (speedup 1.0000x reference)
"""BertSelfAttention Trainium2 kernel.

Full inputs in, full output out. Sharding: 8 cores = (batch b in {0,1}) x
(head-group hg in {0..3}); each core computes 4 heads of one batch and
produces the output feature slice out[b, :, hg*256:(hg+1)*256].

Per-core device program (all cores run the same NEFF, SPMD):
  xT [1024, 2048]      hidden_states[b].T
  QT/KT computed transposed [d, s] (fp32r matmuls), stored fp16 with bias
  V computed [s, d] fp16, rows scaled by exp(mask), plus a per-head
    ones*exp(mask) column so the ctx matmul also yields softmax row sums
  scoresT [k, q] tiles via fp16 matmuls; for each k-tile the two heads'
    MMs are emitted back-to-back on the two PE row-halves (tile_position)
    so each MM's LDWEIGHTS pulls ahead under the other half's stream
  exp on ACT directly from PSUM (scale=1/8, bias=-4 folded in); a tunable
    subset of exps runs on DVE instead via a Schraudolph-style affine
    int16 bit-trick (~±3% on those weights) to keep ACT off the critical
    path
  ctx[q, d] = expT.T @ [V|em] accumulated over 16 k-tiles, then
    per-partition normalize (reciprocal of row sum) + V-bias add on DVE.

ctx-chunk and projection-filler PE work is interleaved BETWEEN scores
batches so the PE never head-of-line blocks on a PSUM slot waiting for
an exp, which keeps the HAM activity monitor from re-throttling the PE
clock to 1.2 GHz mid-kernel.
"""

import numpy as np

B = 2
S = 2048
H = 1024
NH = 16
HD = 64

NCORES = 8
HPC = 4          # heads per core
DS = HPC * HD    # 256 output dims per core
FT = H // 128    # 8 f-tiles (contraction tiles for projections)
KT = S // 128    # 16 key tiles
ST = S // 128    # 16 s-tiles of V
QB = 4           # q blocks of 512
QBS = 512
VW = HPC * (HD + 1)  # 260: V columns + one em column per head

EXP_BIAS = -4.0  # uniform shift inside exp; cancels in softmax, guards fp16

# Schraudolph fp16 exp: bits16(e^(x/8-4)) ~= round(x*SCH_A + SCH_B);
# written through a uint16 view so extreme-negative scores saturate to 0
SCH_A = 1024.0 * 0.125 * np.log2(np.e)
SCH_B = 1024.0 * (15.0 + EXP_BIAS * np.log2(np.e)) - 44.0

# batch indices (0..7) whose B-side exp runs on DVE instead of ACT
DVE_EXP_BATCHES = frozenset({0, 2, 3, 5, 7})

_CACHE = {}


def _build_program(split_waits=True):
    import concourse.bass as bass
    import concourse.mybir as mybir
    import concourse.tile as tile
    from concourse.vector_clock import ScopedClock

    f32 = mybir.dt.float32
    f16 = mybir.dt.float16
    u16 = mybir.dt.uint16
    AF = mybir.ActivationFunctionType
    OP = mybir.AluOpType

    class SplitDrainTileContext(tile.TileContext):
        """The walrus build here rejects instructions with more than one
        sync wait ("Too many sync wait commands"); hoist excess waits onto
        preceding same-engine NOPs."""

        MAX_WAITS_PER_DRAIN = 1
        split_waits_enabled = True

        def _drain_and_barrier(self, tick_clock, wait_clock):
            drain_inst = self.nc.sync.drain()
            wait_clock.add_sem_waits(
                drain_inst.ins, ScopedClock({None: tick_clock.global_clock})
            )
            self.nc.all_engine_barrier()
            assert self.sems is not None
            popped = self.nc._tile_sem_poison_stack.pop()
            assert popped is self._sem_poison
            self.nc.clear_and_free_semaphores(list(self.sems.allocated().values()))
            self.nc.all_engine_barrier()
            if self.split_waits_enabled:
                self._split_multi_waits()

        def _split_multi_waits(self):
            k = self.MAX_WAITS_PER_DRAIN
            nc = self.nc
            for bb in nc.bb_map.values():
                il = bb.bb.instructions
                new = []
                for inst in il:
                    si = getattr(inst, "sync_info", None)
                    waits = list(si.on_wait) if si is not None and si.on_wait else []
                    if len(waits) > k:
                        for j in range(0, len(waits) - k, k):
                            nop = mybir.InstNoOp(
                                name=nc.get_next_instruction_name(),
                                engine=inst.engine,
                                sync_info=mybir.SyncInfo(
                                    on_wait=waits[j : j + k], on_update=[]
                                ),
                                bass_nofuse=True,
                            )
                            new.append(nop)
                        inst.sync_info = mybir.SyncInfo(
                            on_wait=waits[len(waits) - k :],
                            on_update=list(si.on_update) if si.on_update else [],
                        )
                    new.append(inst)
                il[:] = new

    nc = bass.Bass("TRN2", target_bir_lowering=False, debug=False,
                   num_devices=NCORES)

    xT_d = nc.dram_tensor("xT", [H, S], f16, kind="ExternalInput")
    wqT_d = nc.dram_tensor("wqT", [H, DS], f16, kind="ExternalInput")
    wkT_d = nc.dram_tensor("wkT", [H, DS], f16, kind="ExternalInput")
    wvT_d = nc.dram_tensor("wvT", [H, VW], f16, kind="ExternalInput")
    bq_d = nc.dram_tensor("bq", [2, 128, 1], f32, kind="ExternalInput")
    bk_d = nc.dram_tensor("bk", [2, 128, 1], f32, kind="ExternalInput")
    bvb_d = nc.dram_tensor("bvb", [128, DS], f32, kind="ExternalInput")
    em_d = nc.dram_tensor("em", [128, 4 * KT], f32, kind="ExternalInput")
    out_d = nc.dram_tensor("out", [S, DS], f32, kind="ExternalOutput")

    SplitDrainTileContext.split_waits_enabled = split_waits
    with SplitDrainTileContext(nc) as tc:
        from contextlib import ExitStack

        with ExitStack() as ctx:
            const = ctx.enter_context(tc.tile_pool(name="const", bufs=1))
            qk = ctx.enter_context(tc.tile_pool(name="qk", bufs=1))
            vp = ctx.enter_context(tc.tile_pool(name="vp", bufs=1))
            epool = ctx.enter_context(tc.tile_pool(name="epool", bufs=1))
            opool = ctx.enter_context(tc.tile_pool(name="opool", bufs=1))
            rpool = ctx.enter_context(tc.tile_pool(name="rpool", bufs=1))

            # ---- constants ----
            bq_sb = [const.tile([128, 1], f32, tag=f"bq{m}", bufs=1,
                                name=f"bq_sb{m}") for m in range(2)]
            bk_sb = [const.tile([128, 1], f32, tag=f"bk{m}", bufs=1,
                                name=f"bk_sb{m}") for m in range(2)]
            for m in range(2):
                nc.sync.dma_start(bq_sb[m][:], bq_d.ap()[m])
                nc.sync.dma_start(bk_sb[m][:], bk_d.ap()[m])
            bvb_sb = const.tile([128, DS], f32, tag="bvb", bufs=1, name="bvb_sb")
            nc.sync.dma_start(bvb_sb[:], bvb_d.ap())
            em_sb = const.tile([128, 4 * KT], f32, tag="em", bufs=1, name="em_sb")
            nc.sync.dma_start(em_sb[:], em_d.ap())
            ebias = const.tile([128, 1], f32, tag="ebias", bufs=1, name="ebias")
            nc.vector.memset(ebias[:], EXP_BIAS)
            # warm the ACT exp table while DMAs run
            warm = const.tile([128, 1], f32, tag="warm", bufs=1, name="warm")
            nc.scalar.activation(warm[:], ebias[:], AF.Exp)

            # ---- persistent activations ----
            qt = [qk.tile([128, S], f16, tag=f"qt{m}", bufs=1, name=f"qt{m}")
                  for m in range(2)]
            kt_sb = [qk.tile([128, S], f16, tag=f"kt{m}", bufs=1, name=f"kt{m}")
                     for m in range(2)]
            vones = [vp.tile([128, VW], f16, tag=f"v{st}", bufs=1,
                             name=f"vones{st}") for st in range(ST)]

            # ---- input DMAs: batched calls spread over 4 engine queues
            # (per-call issue ~600ns serialized the prologue; each call's
            # descriptors land on one HW DMA queue, so split the
            # prologue-critical tensors 4 ways for parallel bandwidth) ----
            xw = ctx.enter_context(tc.tile_pool(name="xw", bufs=1))
            xt = [xw.tile([128, FT * QBS], f16, tag=f"xt{nb}", bufs=1,
                          name=f"xt{nb}") for nb in range(QB)]
            wq_sb = xw.tile([128, FT * DS], f16, tag="wq", bufs=1, name="wq")
            wk_sb = xw.tile([128, FT * DS], f16, tag="wk", bufs=1, name="wk")
            wv_sb = xw.tile([128, FT * VW], f16, tag="wv", bufs=1, name="wv")
            QUEUES = [nc.sync, nc.scalar, nc.gpsimd]

            def dma_wide(dst_tile, src_dram, width, splits, qoff=0):
                g = FT // splits
                for i in range(splits):
                    fs = slice(i * g * 128, (i + 1) * g * 128)
                    cs = slice(i * g * width, (i + 1) * g * width)
                    QUEUES[(qoff + i) % 3].dma_start(
                        dst_tile[:, cs].rearrange("p (f d) -> p f d", f=g),
                        src_dram[fs, :].rearrange("(f p) d -> p f d", f=g))

            # prologue-critical: wk, xt0, wq issued from sync (whose HW
            # queues start draining earliest), each split on its own queue
            def dma_sync(dst_tile, src_dram, width, splits):
                g = FT // splits
                for i in range(splits):
                    fs = slice(i * g * 128, (i + 1) * g * 128)
                    cs = slice(i * g * width, (i + 1) * g * width)
                    nc.sync.dma_start(
                        dst_tile[:, cs].rearrange("p (f d) -> p f d", f=g),
                        src_dram[fs, :].rearrange("(f p) d -> p f d", f=g))

            dma_sync(wk_sb[:], wkT_d.ap(), DS, 4)
            dma_sync(xt[0][:], xT_d.ap()[:, 0:QBS], QBS, 4)
            dma_sync(wq_sb[:], wqT_d.ap(), DS, 4)
            for nb in range(1, QB):
                ns = slice(nb * QBS, (nb + 1) * QBS)
                dma_wide(xt[nb][:], xT_d.ap()[:, ns], QBS, 2, nb + 1)
            dma_wide(wv_sb[:], wvT_d.ap(), VW, 2, 0)

            # ---- PSUM pools: proj 1 + scores 3x2 + ctx 1 = 8 banks ----
            ps_pj = ctx.enter_context(
                tc.tile_pool(name="ps_pj", bufs=1, space="PSUM"))
            ps_sc = ctx.enter_context(
                tc.tile_pool(name="ps_sc", bufs=3, space="PSUM"))
            ps_cx = ctx.enter_context(
                tc.tile_pool(name="ps_cx", bufs=1, space="PSUM"))

            mm = nc.tensor.matmul

            # ---- work units (each emits PE work + its evictions) ----
            def qk_proj_block(w_sb, bias_sb, dst, m, nb):
                ns = slice(nb * QBS, (nb + 1) * QBS)
                ps = ps_pj.tile([128, QBS], f32, tag="pj", name="pspj")
                for ft in range(FT):
                    mm(ps[:],
                       w_sb[:, ft * DS + m * 128:ft * DS + (m + 1) * 128],
                       xt[nb][:, ft * QBS:(ft + 1) * QBS],
                       start=(ft == 0), stop=(ft == FT - 1))
                nc.vector.tensor_scalar_add(dst[:, ns], ps[:], bias_sb[:])

            def v_proj_block(st):
                nb, within = divmod(st, 4)
                ps = ps_pj.tile([128, QBS], f32, tag="pj", name="pspjv")
                for ft in range(FT):
                    lo = ft * QBS + within * 128
                    mm(ps[:, 0:VW],
                       xt[nb][:, lo:lo + 128],
                       wv_sb[:, ft * VW:(ft + 1) * VW],
                       start=(ft == 0), stop=(ft == FT - 1))
                nc.vector.tensor_scalar_mul(
                    vones[st][:], ps[:, 0:VW], em_sb[:, 4 * st:4 * st + 1])
                # all 4 per-head em columns in one strided copy
                nc.vector.tensor_copy(
                    vones[st][:][:, HD::HD + 1],
                    em_sb[:, 4 * st:4 * st + 4])

            BATCHES = [(0, 2), (2, 2), (4, 2), (6, 2), (8, 2), (10, 2),
                       (12, 2), (14, 2)]

            def scores_batch(hp, qb, eA, eB, k0, nk, dve_b=False):
                """Per k-tile: the two heads' score MMs back-to-back on the
                two PE row-halves; each MM's LDWEIGHTS pulls ahead under the
                other half's stream (row groups don't conflict)."""
                qs = slice(qb * QBS, (qb + 1) * QBS)
                w = nk * QBS
                es = slice(k0 * QBS, k0 * QBS + w)
                psA = ps_sc.tile([128, 2 * QBS], f32, tag="sc", name="pscA")
                psB = ps_sc.tile([128, 2 * QBS], f32, tag="sc", name="pscB")
                last_mm = None
                for j in range(nk):
                    ktile = k0 + j
                    ks = slice(ktile * 128, (ktile + 1) * 128)
                    js = slice(j * QBS, (j + 1) * QBS)
                    for (ps, p0) in ((psA, 0), (psB, 64)):
                        last_mm = mm(ps[:, js],
                                     kt_sb[hp][p0:p0 + 64, ks],
                                     qt[hp][p0:p0 + 64, qs],
                                     tile_position=(p0, 0))
                nc.scalar.activation(eA[:, es], psA[:, 0:w],
                                     AF.Exp, bias=ebias[:], scale=0.125)
                if dve_b:
                    nc.vector.tensor_scalar(
                        eB[:, es].bitcast(u16), psB[:, 0:w],
                        SCH_A, SCH_B, op0=OP.mult, op1=OP.add)
                else:
                    nc.scalar.activation(eB[:, es], psB[:, 0:w],
                                         AF.Exp, bias=ebias[:], scale=0.125)

            def ctx_chunks(prev_state):
                """One iteration's ctx work as 8 thunks (2 heads x 4 q-tiles
                of 128), to be interleaved between the next iteration's
                scores batches."""
                hp, qb, eA, eB = prev_state
                qtile0 = qb * 4
                state = {"ots": []}
                chunks = []

                def mk_chunk(a, e, qq):
                    def chunk():
                        if qq == 0:
                            state["cps"] = ps_cx.tile(
                                [128, 4 * (HD + 1)], f32, tag="cx", name="cps")
                        hh = 2 * hp + a
                        cpsb = state["cps"]
                        cps = cpsb[:, qq * (HD + 1):(qq + 1) * (HD + 1)]
                        for ktile in range(KT):
                            lo = ktile * QBS + qq * 128
                            mm(cps,
                               e[:, lo:lo + 128],
                               vones[ktile][:, hh * (HD + 1):(hh + 1) * (HD + 1)],
                               start=(ktile == 0), stop=(ktile == KT - 1))
                        if qq == 3:
                            # evict all 4 q-tiles: batched reciprocal of the
                            # row-sum columns, then normalize + V-bias add
                            r = rpool.tile([128, 4], f32, tag="r", bufs=2,
                                           name="r")
                            nc.vector.reciprocal(
                                r[:], cpsb[:][:, HD::HD + 1])
                            for q2 in range(4):
                                cps2 = cpsb[:, q2 * (HD + 1):
                                            q2 * (HD + 1) + HD]
                                if a == 0:
                                    ot = opool.tile([128, 128], f32, tag="ot",
                                                    bufs=4, name="ot")
                                    state["ots"].append(ot)
                                else:
                                    ot = state["ots"][q2]
                                nc.vector.scalar_tensor_tensor(
                                    ot[:, a * 64:(a + 1) * 64],
                                    cps2, r[:, q2:q2 + 1],
                                    bvb_sb[:, hh * HD:(hh + 1) * HD],
                                    op0=OP.mult, op1=OP.add)
                                if a == 1:
                                    qt_idx = qtile0 + q2
                                    nc.sync.dma_start(
                                        out_d.ap()[qt_idx * 128:
                                                   (qt_idx + 1) * 128,
                                                   hp * 128:(hp + 1) * 128],
                                        ot[:])
                    return chunk

                for a, e in ((0, eA), (1, eB)):
                    for qq in range(4):
                        chunks.append(mk_chunk(a, e, qq))
                return chunks

            # ---- emission schedule ----
            def k0_block(nb):
                qk_proj_block(wk_sb, bk_sb[0], kt_sb[0], 0, nb)

            def q0_block(nb):
                qk_proj_block(wq_sb, bq_sb[0], qt[0], 0, nb)

            def k1_block(nb):
                qk_proj_block(wk_sb, bk_sb[1], kt_sb[1], 1, nb)

            def q1_block(nb):
                qk_proj_block(wq_sb, bq_sb[1], qt[1], 1, nb)

            # minimal prologue: it 0 only needs K m0 k-tiles 0/1 (batch bi
            # reads k-tiles 2bi..2bi+1, K-block nb covers k-tiles 4nb..4nb+3
            # so K-nb0 covers batches 0-1) and Q m0 q-block 0
            k0_block(0)
            q0_block(0)

            # filler units per attention iteration; each must be emitted
            # before the scores batch that first reads its output (K-m0
            # block nb is first read by batch 2nb of it 0 — the
            # proportional interleave below places work[i] early enough)
            fillers = {
                0: [lambda: k0_block(1), lambda: k0_block(2),
                    lambda: k0_block(3), lambda: q0_block(1)]
                   + [lambda st=st: v_proj_block(st) for st in range(6)],
                1: [lambda: q0_block(2)]
                   + [lambda st=st: v_proj_block(st) for st in range(6, ST)],
                2: [lambda: q0_block(3), lambda: k1_block(0),
                    lambda: k1_block(1)],
                3: [lambda: k1_block(2), lambda: k1_block(3),
                    lambda: q1_block(0)],
                4: [lambda: q1_block(1)],
                5: [lambda: q1_block(2)],
                6: [lambda: q1_block(3)],
            }

            prev = None
            for it in range(8):
                hp, qb = divmod(it, QB)
                eA = epool.tile([128, KT * QBS], f16, tag="eA", bufs=3,
                                name="eA")
                eB = epool.tile([128, KT * QBS], f16, tag="eB", bufs=3,
                                name="eB")
                # fillers BEFORE ctx chunks: iteration 1's ctx (for it 0)
                # reads vones[8..15], which fillers[1] produces
                work = list(fillers.get(it, []))
                if prev is not None:
                    work.extend(ctx_chunks(prev))
                done = 0
                for bi, (k0, nk) in enumerate(BATCHES):
                    scores_batch(hp, qb, eA, eB, k0, nk,
                                 dve_b=(bi in DVE_EXP_BATCHES))
                    end = (bi + 1) * len(work) // len(BATCHES)
                    while done < end:
                        work[done]()
                        done += 1
                prev = (hp, qb, eA, eB)
            for chunk in ctx_chunks(prev):
                chunk()

    return nc


def _get_program(split_waits=True):
    key = ("nc", split_waits)
    if key not in _CACHE:
        _CACHE[key] = _build_program(split_waits)
    return _CACHE[key]


def _make_in_maps(hidden_states, attention_mask, Wq, bq, Wk, bk, Wv, bv):
    hidden = np.ascontiguousarray(np.asarray(hidden_states, dtype=np.float32))
    mask = np.asarray(attention_mask, dtype=np.float32)
    Wq = np.asarray(Wq, dtype=np.float32)
    Wk = np.asarray(Wk, dtype=np.float32)
    Wv = np.asarray(Wv, dtype=np.float32)
    bq = np.asarray(bq, dtype=np.float32)
    bk = np.asarray(bk, dtype=np.float32)
    bv = np.asarray(bv, dtype=np.float32)

    WqT = Wq.T  # [in, out]
    WkT = Wk.T
    WvT = Wv.T

    in_maps = []
    for c in range(NCORES):
        b, hg = divmod(c, HPC)
        cols = slice(hg * DS, (hg + 1) * DS)
        xT = np.ascontiguousarray(hidden[b].T.astype(np.float16))
        wqT = np.ascontiguousarray(WqT[:, cols].astype(np.float16))
        wkT = np.ascontiguousarray(WkT[:, cols].astype(np.float16))
        wv_base = WvT[:, cols]
        wvT = np.zeros((H, VW), np.float16)
        for hh in range(HPC):
            wvT[:, hh * (HD + 1):hh * (HD + 1) + HD] = \
                wv_base[:, hh * HD:(hh + 1) * HD]
        bq_c = np.ascontiguousarray(bq[cols].reshape(2, 128, 1))
        bk_c = np.ascontiguousarray(bk[cols].reshape(2, 128, 1))
        bvb = np.ascontiguousarray(np.tile(bv[cols][None, :], (128, 1)))
        em = np.exp(mask[b, 0, 0, :]).reshape(KT, 128).T.astype(np.float32)
        em4 = np.ascontiguousarray(np.repeat(em, 4, axis=1))
        in_maps.append({
            "xT": xT, "wqT": wqT, "wkT": wkT, "wvT": wvT,
            "bq": bq_c, "bk": bk_c, "bvb": bvb, "em": em4,
        })
    return in_maps


def _assemble(results):
    out = np.empty((B, S, H), np.float32)
    for c in range(NCORES):
        b, hg = divmod(c, HPC)
        out[b][:, hg * DS:(hg + 1) * DS] = results[c]["out"]
    return out


def _run(in_maps, trace=False):
    from concourse.bass_utils import run_bass_kernel_spmd
    nc = _get_program()
    return run_bass_kernel_spmd(
        nc, in_maps, core_ids=list(range(NCORES)), trace=trace)


def kernel(**inputs):
    in_maps = _make_in_maps(**inputs)
    res = _run(in_maps, trace=False)
    return _assemble(res.results)


# revision 33
# speedup vs baseline: 1.1494x; 1.1494x over previous
"""BertSelfAttention Trainium2 kernel.

Full inputs in, full output out. Sharding: 8 cores = (batch b in {0,1}) x
(head-group hg in {0..3}); each core computes 4 heads of one batch and
produces the output feature slice out[b, :, hg*256:(hg+1)*256].

Per-core device program (all cores run the same NEFF, SPMD):
  xT [1024, 2048]      hidden_states[b].T
  QT/KT computed transposed [d, s] (fp32r matmuls), stored fp16 with bias
  V computed [s, d] fp16, rows scaled by exp(mask), plus a per-head
    ones*exp(mask) column so the ctx matmul also yields softmax row sums
  scoresT [k, q] tiles via fp16 matmuls; for each k-tile the two heads'
    MMs are emitted back-to-back on the two PE row-halves (tile_position)
    so each MM's LDWEIGHTS pulls ahead under the other half's stream
  exp on ACT directly from PSUM (scale=1/8, bias=-4 folded in); a tunable
    subset of exps runs on DVE instead via a Schraudolph-style affine
    int16 bit-trick (~±3% on those weights) to keep ACT off the critical
    path
  ctx[q, d] = expT.T @ [V|em] accumulated over 16 k-tiles, then
    per-partition normalize (reciprocal of row sum) + V-bias add on DVE.

ctx-chunk and projection-filler PE work is interleaved BETWEEN scores
batches so the PE never head-of-line blocks on a PSUM slot waiting for
an exp, which keeps the HAM activity monitor from re-throttling the PE
clock to 1.2 GHz mid-kernel.
"""

import numpy as np

B = 2
S = 2048
H = 1024
NH = 16
HD = 64

NCORES = 8
HPC = 4          # heads per core
DS = HPC * HD    # 256 output dims per core
FT = H // 128    # 8 f-tiles (contraction tiles for projections)
KT = S // 128    # 16 key tiles
ST = S // 128    # 16 s-tiles of V
QB = 4           # q blocks of 512
QBS = 512
VW = HPC * (HD + 1)  # 260: V columns + one em column per head

EXP_BIAS = -4.0  # uniform shift inside exp; cancels in softmax, guards fp16

# Schraudolph fp16 exp: bits16(e^(x/8-4)) ~= round(x*SCH_A + SCH_B);
# written through a uint16 view so extreme-negative scores saturate to 0
SCH_A = 1024.0 * 0.125 * np.log2(np.e)
SCH_B = 1024.0 * (15.0 + EXP_BIAS * np.log2(np.e)) - 44.0

# batch indices (0..7) whose B-side exp runs on DVE instead of ACT
DVE_EXP_BATCHES = frozenset({0, 2, 3, 5, 7})

_CACHE = {}


def _build_program(split_waits=True):
    import concourse.bass as bass
    import concourse.mybir as mybir
    import concourse.tile as tile
    from concourse.vector_clock import ScopedClock

    f32 = mybir.dt.float32
    f16 = mybir.dt.float16
    u16 = mybir.dt.uint16
    AF = mybir.ActivationFunctionType
    OP = mybir.AluOpType

    class SplitDrainTileContext(tile.TileContext):
        """The walrus build here rejects instructions with more than one
        sync wait ("Too many sync wait commands"); hoist excess waits onto
        preceding same-engine NOPs."""

        MAX_WAITS_PER_DRAIN = 1
        split_waits_enabled = True

        def _drain_and_barrier(self, tick_clock, wait_clock):
            drain_inst = self.nc.sync.drain()
            wait_clock.add_sem_waits(
                drain_inst.ins, ScopedClock({None: tick_clock.global_clock})
            )
            self.nc.all_engine_barrier()
            assert self.sems is not None
            popped = self.nc._tile_sem_poison_stack.pop()
            assert popped is self._sem_poison
            self.nc.clear_and_free_semaphores(list(self.sems.allocated().values()))
            self.nc.all_engine_barrier()
            if self.split_waits_enabled:
                self._split_multi_waits()

        def _split_multi_waits(self):
            k = self.MAX_WAITS_PER_DRAIN
            nc = self.nc
            for bb in nc.bb_map.values():
                il = bb.bb.instructions
                new = []
                for inst in il:
                    si = getattr(inst, "sync_info", None)
                    waits = list(si.on_wait) if si is not None and si.on_wait else []
                    if len(waits) > k:
                        for j in range(0, len(waits) - k, k):
                            nop = mybir.InstNoOp(
                                name=nc.get_next_instruction_name(),
                                engine=inst.engine,
                                sync_info=mybir.SyncInfo(
                                    on_wait=waits[j : j + k], on_update=[]
                                ),
                                bass_nofuse=True,
                            )
                            new.append(nop)
                        inst.sync_info = mybir.SyncInfo(
                            on_wait=waits[len(waits) - k :],
                            on_update=list(si.on_update) if si.on_update else [],
                        )
                    new.append(inst)
                il[:] = new

    nc = bass.Bass("TRN2", target_bir_lowering=False, debug=False,
                   num_devices=NCORES)

    xT_d = nc.dram_tensor("xT", [H, S], f16, kind="ExternalInput")
    wqT_d = nc.dram_tensor("wqT", [H, DS], f16, kind="ExternalInput")
    wkT_d = nc.dram_tensor("wkT", [H, DS], f16, kind="ExternalInput")
    wvT_d = nc.dram_tensor("wvT", [H, VW], f16, kind="ExternalInput")
    bq_d = nc.dram_tensor("bq", [2, 128, 1], f32, kind="ExternalInput")
    bk_d = nc.dram_tensor("bk", [2, 128, 1], f32, kind="ExternalInput")
    bvb_d = nc.dram_tensor("bvb", [128, DS], f32, kind="ExternalInput")
    em_d = nc.dram_tensor("em", [128, 4 * KT], f32, kind="ExternalInput")
    out_d = nc.dram_tensor("out", [S, DS], f32, kind="ExternalOutput")

    SplitDrainTileContext.split_waits_enabled = split_waits
    with SplitDrainTileContext(nc) as tc:
        from contextlib import ExitStack

        with ExitStack() as ctx:
            const = ctx.enter_context(tc.tile_pool(name="const", bufs=1))
            qk = ctx.enter_context(tc.tile_pool(name="qk", bufs=1))
            vp = ctx.enter_context(tc.tile_pool(name="vp", bufs=1))
            epool = ctx.enter_context(tc.tile_pool(name="epool", bufs=1))
            opool = ctx.enter_context(tc.tile_pool(name="opool", bufs=1))
            rpool = ctx.enter_context(tc.tile_pool(name="rpool", bufs=1))

            # ---- constants ----
            bq_sb = [const.tile([128, 1], f32, tag=f"bq{m}", bufs=1,
                                name=f"bq_sb{m}") for m in range(2)]
            bk_sb = [const.tile([128, 1], f32, tag=f"bk{m}", bufs=1,
                                name=f"bk_sb{m}") for m in range(2)]
            for m in range(2):
                nc.sync.dma_start(bq_sb[m][:], bq_d.ap()[m])
                nc.sync.dma_start(bk_sb[m][:], bk_d.ap()[m])
            bvb_sb = const.tile([128, DS], f32, tag="bvb", bufs=1, name="bvb_sb")
            nc.sync.dma_start(bvb_sb[:], bvb_d.ap())
            em_sb = const.tile([128, 4 * KT], f32, tag="em", bufs=1, name="em_sb")
            nc.sync.dma_start(em_sb[:], em_d.ap())
            ebias = const.tile([128, 1], f32, tag="ebias", bufs=1, name="ebias")
            nc.vector.memset(ebias[:], EXP_BIAS)
            # warm the ACT exp table while DMAs run
            warm = const.tile([128, 1], f32, tag="warm", bufs=1, name="warm")
            nc.scalar.activation(warm[:], ebias[:], AF.Exp)

            # ---- persistent activations ----
            qt = [qk.tile([128, S], f16, tag=f"qt{m}", bufs=1, name=f"qt{m}")
                  for m in range(2)]
            kt_sb = [qk.tile([128, S], f16, tag=f"kt{m}", bufs=1, name=f"kt{m}")
                     for m in range(2)]
            vones = [vp.tile([128, VW], f16, tag=f"v{st}", bufs=1,
                             name=f"vones{st}") for st in range(ST)]

            # ---- input DMAs: batched calls spread over 4 engine queues
            # (per-call issue ~600ns serialized the prologue; each call's
            # descriptors land on one HW DMA queue, so split the
            # prologue-critical tensors 4 ways for parallel bandwidth) ----
            # Prologue-critical tensors are split into per-DMA TILES so
            # the first projection matmuls' read deps cover only the tile
            # they touch (tile-granular dep tracking otherwise stalls the
            # first MM until the whole tensor lands, ~25us).
            xw = ctx.enter_context(tc.tile_pool(name="xw", bufs=1))
            # xt[0]: 4 tiles x 2 ft; xt[1..3]: 2 tiles x 4 ft
            xt_tiles = []
            xt_per = []
            for nb in range(QB):
                ntile = 4 if nb == 0 else 2
                per = FT // ntile
                xt_per.append(per)
                xt_tiles.append([
                    xw.tile([128, per * QBS], f16, tag=f"xt{nb}_{i}", bufs=1,
                            name=f"xt{nb}_{i}") for i in range(ntile)])
            wq_sb = [xw.tile([128, 2 * DS], f16, tag=f"wq{i}", bufs=1,
                             name=f"wq{i}") for i in range(4)]
            wk_sb = [xw.tile([128, 2 * DS], f16, tag=f"wk{i}", bufs=1,
                             name=f"wk{i}") for i in range(4)]
            wv_sb = xw.tile([128, FT * VW], f16, tag="wv", bufs=1, name="wv")

            def xt_ap(nb, ft, off, width):
                per = xt_per[nb]
                t = xt_tiles[nb][ft // per]
                lo = (ft % per) * QBS + off
                return t[:, lo:lo + width]

            def w_ap(w_tiles, ft, off, width):
                t = w_tiles[ft // 2]
                lo = (ft % 2) * DS + off
                return t[:, lo:lo + width]

            def dma_tile(eng, dst_tile, src_dram, g):
                eng.dma_start(
                    dst_tile[:].rearrange("p (f d) -> p f d", f=g),
                    src_dram.rearrange("(f p) d -> p f d", f=g))

            # prologue-critical from sync (whose HW queues drain earliest)
            for i in range(4):
                dma_tile(nc.sync, wk_sb[i],
                         wkT_d.ap()[i * 256:(i + 1) * 256, :], 2)
            for i in range(4):
                dma_tile(nc.sync, xt_tiles[0][i],
                         xT_d.ap()[i * 256:(i + 1) * 256, 0:QBS], 2)
            for i in range(4):
                dma_tile(nc.sync, wq_sb[i],
                         wqT_d.ap()[i * 256:(i + 1) * 256, :], 2)
            QUEUES = [nc.scalar, nc.gpsimd]
            qn = 0
            for nb in range(1, QB):
                ns = slice(nb * QBS, (nb + 1) * QBS)
                for i in range(2):
                    dma_tile(QUEUES[qn % 2], xt_tiles[nb][i],
                             xT_d.ap()[i * 512:(i + 1) * 512, ns], 4)
                    qn += 1
            nc.gpsimd.dma_start(
                wv_sb[:].rearrange("p (f d) -> p f d", f=FT),
                wvT_d.ap().rearrange("(f p) d -> p f d", f=FT))

            # ---- PSUM pools: proj 1 + scores 3x2 + ctx 1 = 8 banks ----
            ps_pj = ctx.enter_context(
                tc.tile_pool(name="ps_pj", bufs=1, space="PSUM"))
            ps_sc = ctx.enter_context(
                tc.tile_pool(name="ps_sc", bufs=3, space="PSUM"))
            ps_cx = ctx.enter_context(
                tc.tile_pool(name="ps_cx", bufs=1, space="PSUM"))

            mm = nc.tensor.matmul

            # ---- work units (each emits PE work + its evictions) ----
            def qk_proj_block(w_sb, bias_sb, dst, m, nb):
                ns = slice(nb * QBS, (nb + 1) * QBS)
                ps = ps_pj.tile([128, QBS], f32, tag="pj", name="pspj")
                for ft in range(FT):
                    mm(ps[:],
                       w_ap(w_sb, ft, m * 128, 128),
                       xt_ap(nb, ft, 0, QBS),
                       start=(ft == 0), stop=(ft == FT - 1))
                nc.vector.tensor_scalar_add(dst[:, ns], ps[:], bias_sb[:])

            def v_proj_block(st):
                nb, within = divmod(st, 4)
                ps = ps_pj.tile([128, QBS], f32, tag="pj", name="pspjv")
                for ft in range(FT):
                    mm(ps[:, 0:VW],
                       xt_ap(nb, ft, within * 128, 128),
                       wv_sb[:, ft * VW:(ft + 1) * VW],
                       start=(ft == 0), stop=(ft == FT - 1))
                nc.vector.tensor_scalar_mul(
                    vones[st][:], ps[:, 0:VW], em_sb[:, 4 * st:4 * st + 1])
                # all 4 per-head em columns in one strided copy
                nc.vector.tensor_copy(
                    vones[st][:][:, HD::HD + 1],
                    em_sb[:, 4 * st:4 * st + 4])

            BATCHES = [(0, 2), (2, 2), (4, 2), (6, 2), (8, 2), (10, 2),
                       (12, 2), (14, 2)]

            def scores_batch(hp, qb, eA, eB, k0, nk, dve_b=False):
                """Per k-tile: the two heads' score MMs back-to-back on the
                two PE row-halves; each MM's LDWEIGHTS pulls ahead under the
                other half's stream (row groups don't conflict)."""
                qs = slice(qb * QBS, (qb + 1) * QBS)
                w = nk * QBS
                es = slice(k0 * QBS, k0 * QBS + w)
                psA = ps_sc.tile([128, 2 * QBS], f32, tag="sc", name="pscA")
                psB = ps_sc.tile([128, 2 * QBS], f32, tag="sc", name="pscB")
                last_mm = None
                for j in range(nk):
                    ktile = k0 + j
                    ks = slice(ktile * 128, (ktile + 1) * 128)
                    js = slice(j * QBS, (j + 1) * QBS)
                    for (ps, p0) in ((psA, 0), (psB, 64)):
                        last_mm = mm(ps[:, js],
                                     kt_sb[hp][p0:p0 + 64, ks],
                                     qt[hp][p0:p0 + 64, qs],
                                     tile_position=(p0, 0))
                nc.scalar.activation(eA[:, es], psA[:, 0:w],
                                     AF.Exp, bias=ebias[:], scale=0.125)
                if dve_b:
                    nc.vector.tensor_scalar(
                        eB[:, es].bitcast(u16), psB[:, 0:w],
                        SCH_A, SCH_B, op0=OP.mult, op1=OP.add)
                else:
                    nc.scalar.activation(eB[:, es], psB[:, 0:w],
                                         AF.Exp, bias=ebias[:], scale=0.125)

            def ctx_chunks(prev_state):
                """One iteration's ctx work as 8 thunks (2 heads x 4 q-tiles
                of 128), to be interleaved between the next iteration's
                scores batches."""
                hp, qb, eA, eB = prev_state
                qtile0 = qb * 4
                state = {"ots": []}
                chunks = []

                def mk_chunk(a, e, qq):
                    def chunk():
                        if qq == 0:
                            state["cps"] = ps_cx.tile(
                                [128, 4 * (HD + 1)], f32, tag="cx", name="cps")
                        hh = 2 * hp + a
                        cpsb = state["cps"]
                        cps = cpsb[:, qq * (HD + 1):(qq + 1) * (HD + 1)]
                        for ktile in range(KT):
                            lo = ktile * QBS + qq * 128
                            mm(cps,
                               e[:, lo:lo + 128],
                               vones[ktile][:, hh * (HD + 1):(hh + 1) * (HD + 1)],
                               start=(ktile == 0), stop=(ktile == KT - 1))
                        if qq == 3:
                            # evict all 4 q-tiles: batched reciprocal of the
                            # row-sum columns, then normalize + V-bias add
                            r = rpool.tile([128, 4], f32, tag="r", bufs=2,
                                           name="r")
                            nc.vector.reciprocal(
                                r[:], cpsb[:][:, HD::HD + 1])
                            for q2 in range(4):
                                cps2 = cpsb[:, q2 * (HD + 1):
                                            q2 * (HD + 1) + HD]
                                if a == 0:
                                    ot = opool.tile([128, 128], f32, tag="ot",
                                                    bufs=4, name="ot")
                                    state["ots"].append(ot)
                                else:
                                    ot = state["ots"][q2]
                                nc.vector.scalar_tensor_tensor(
                                    ot[:, a * 64:(a + 1) * 64],
                                    cps2, r[:, q2:q2 + 1],
                                    bvb_sb[:, hh * HD:(hh + 1) * HD],
                                    op0=OP.mult, op1=OP.add)
                                if a == 1:
                                    qt_idx = qtile0 + q2
                                    nc.sync.dma_start(
                                        out_d.ap()[qt_idx * 128:
                                                   (qt_idx + 1) * 128,
                                                   hp * 128:(hp + 1) * 128],
                                        ot[:])
                    return chunk

                for a, e in ((0, eA), (1, eB)):
                    for qq in range(4):
                        chunks.append(mk_chunk(a, e, qq))
                return chunks

            # ---- emission schedule ----
            def k0_block(nb):
                qk_proj_block(wk_sb, bk_sb[0], kt_sb[0], 0, nb)

            def q0_block(nb):
                qk_proj_block(wq_sb, bq_sb[0], qt[0], 0, nb)

            def k1_block(nb):
                qk_proj_block(wk_sb, bk_sb[1], kt_sb[1], 1, nb)

            def q1_block(nb):
                qk_proj_block(wq_sb, bq_sb[1], qt[1], 1, nb)

            # minimal prologue: it 0 only needs K m0 k-tiles 0/1 (batch bi
            # reads k-tiles 2bi..2bi+1, K-block nb covers k-tiles 4nb..4nb+3
            # so K-nb0 covers batches 0-1) and Q m0 q-block 0
            k0_block(0)
            q0_block(0)

            # filler units per attention iteration; each must be emitted
            # before the scores batch that first reads its output (K-m0
            # block nb is first read by batch 2nb of it 0 — the
            # proportional interleave below places work[i] early enough)
            fillers = {
                0: [lambda: k0_block(1), lambda: k0_block(2),
                    lambda: k0_block(3), lambda: q0_block(1)]
                   + [lambda st=st: v_proj_block(st) for st in range(6)],
                1: [lambda: q0_block(2)]
                   + [lambda st=st: v_proj_block(st) for st in range(6, ST)],
                2: [lambda: q0_block(3), lambda: k1_block(0),
                    lambda: k1_block(1)],
                3: [lambda: k1_block(2), lambda: k1_block(3),
                    lambda: q1_block(0)],
                4: [lambda: q1_block(1)],
                5: [lambda: q1_block(2)],
                6: [lambda: q1_block(3)],
            }

            prev = None
            for it in range(8):
                hp, qb = divmod(it, QB)
                eA = epool.tile([128, KT * QBS], f16, tag="eA", bufs=3,
                                name="eA")
                eB = epool.tile([128, KT * QBS], f16, tag="eB", bufs=3,
                                name="eB")
                # fillers BEFORE ctx chunks: iteration 1's ctx (for it 0)
                # reads vones[8..15], which fillers[1] produces
                work = list(fillers.get(it, []))
                if prev is not None:
                    work.extend(ctx_chunks(prev))
                done = 0
                for bi, (k0, nk) in enumerate(BATCHES):
                    scores_batch(hp, qb, eA, eB, k0, nk,
                                 dve_b=(bi in DVE_EXP_BATCHES))
                    end = (bi + 1) * len(work) // len(BATCHES)
                    while done < end:
                        work[done]()
                        done += 1
                prev = (hp, qb, eA, eB)
            for chunk in ctx_chunks(prev):
                chunk()

    return nc


def _get_program(split_waits=True):
    key = ("nc", split_waits)
    if key not in _CACHE:
        _CACHE[key] = _build_program(split_waits)
    return _CACHE[key]


def _make_in_maps(hidden_states, attention_mask, Wq, bq, Wk, bk, Wv, bv):
    hidden = np.ascontiguousarray(np.asarray(hidden_states, dtype=np.float32))
    mask = np.asarray(attention_mask, dtype=np.float32)
    Wq = np.asarray(Wq, dtype=np.float32)
    Wk = np.asarray(Wk, dtype=np.float32)
    Wv = np.asarray(Wv, dtype=np.float32)
    bq = np.asarray(bq, dtype=np.float32)
    bk = np.asarray(bk, dtype=np.float32)
    bv = np.asarray(bv, dtype=np.float32)

    WqT = Wq.T  # [in, out]
    WkT = Wk.T
    WvT = Wv.T

    in_maps = []
    for c in range(NCORES):
        b, hg = divmod(c, HPC)
        cols = slice(hg * DS, (hg + 1) * DS)
        xT = np.ascontiguousarray(hidden[b].T.astype(np.float16))
        wqT = np.ascontiguousarray(WqT[:, cols].astype(np.float16))
        wkT = np.ascontiguousarray(WkT[:, cols].astype(np.float16))
        wv_base = WvT[:, cols]
        wvT = np.zeros((H, VW), np.float16)
        for hh in range(HPC):
            wvT[:, hh * (HD + 1):hh * (HD + 1) + HD] = \
                wv_base[:, hh * HD:(hh + 1) * HD]
        bq_c = np.ascontiguousarray(bq[cols].reshape(2, 128, 1))
        bk_c = np.ascontiguousarray(bk[cols].reshape(2, 128, 1))
        bvb = np.ascontiguousarray(np.tile(bv[cols][None, :], (128, 1)))
        em = np.exp(mask[b, 0, 0, :]).reshape(KT, 128).T.astype(np.float32)
        em4 = np.ascontiguousarray(np.repeat(em, 4, axis=1))
        in_maps.append({
            "xT": xT, "wqT": wqT, "wkT": wkT, "wvT": wvT,
            "bq": bq_c, "bk": bk_c, "bvb": bvb, "em": em4,
        })
    return in_maps


def _assemble(results):
    out = np.empty((B, S, H), np.float32)
    for c in range(NCORES):
        b, hg = divmod(c, HPC)
        out[b][:, hg * DS:(hg + 1) * DS] = results[c]["out"]
    return out


def _run(in_maps, trace=False):
    from concourse.bass_utils import run_bass_kernel_spmd
    nc = _get_program()
    return run_bass_kernel_spmd(
        nc, in_maps, core_ids=list(range(NCORES)), trace=trace)


def kernel(**inputs):
    in_maps = _make_in_maps(**inputs)
    res = _run(in_maps, trace=False)
    return _assemble(res.results)
